# revision 1
# baseline (speedup 1.0000x reference)
"""Trainium2 Bass kernel for nn_DAGModel (gnn_message_passing).

Strategy (data-parallel over batch, 8 b's per core):
- node_vecs live in DRAM as a bf16 table `nv[token, b8, h128]` (2KB rows).
- Parent gathers use GPSIMD dma_gather(transpose=True) in PREPARE_ONLY
  mode + trigger_dma: the gpsimd only generates descriptors (~1.1us) and
  the 2KB-row transfers run asynchronously on the DMA engines, so the
  Pool engine is no longer serialized on gather transfer time.
- Nodes of each depth are reordered host-side by (has-parent-in-previous-
  depth, parent-count desc). Chunks whose parents all come from older
  depths bound their gather source AP below the previous depth's slab, so
  those gathers overlap the previous depth's tail compute/writeback.
- The parent-slot sum accumulates IN PLACE into the slot-0 segment of the
  gathered tile (bf16 adds on DVE), which doubles as the MLP rhs `pv`.
- The 2-layer MLP runs in bf16 on the PE (f32 PSUM accumulate); ReLU+b1
  and +b2 ride the Scalar engine's activation; the residual +pv is a DVE
  bf16 add.
- Output projection out[t] = nv[t]·Wout[t] is computed feature-major:
  m = nvn * woutT elementwise (DVE, bf16), then a ones-vector matmul
  reduces over partitions into PSUM rows (per b-pair at partition 32*bp).
- new vecs are PE-transposed (bf16) back to row-major and DMA'd to the
  next depth's token rows.
"""

import numpy as np
import ml_dtypes

BF16 = ml_dtypes.bfloat16

# Full-problem dims (hardcoded per contract).
B, H, E = 64, 128, 128
D_FULL, P_FULL, MP = 20, 1000, 8
NCORES, BL = 8, 8
BCAP = 512  # SWDGE ring is ~512 descs/dir
LAST_RESULTS = None


# ---------------------------------------------------------------------------
# workaround: this walrus build rejects >1 sync-wait on a CTRL (Drain) inst.
def _install_tilefix():
    import concourse.tile as tile_mod
    from concourse.vector_clock import ScopedClock, VectorClock

    if getattr(tile_mod.TileContext, "_drain_split_installed", False):
        return

    def _split_drain_and_barrier(self, tick_clock, wait_clock):
        gc = tick_clock.global_clock
        ticks = list(gc)
        nz = [(i, t) for i, t in enumerate(ticks) if t > 0]
        if nz:
            for i, t in nz:
                vec = [0] * len(ticks)
                vec[i] = t
                d = self.nc.sync.drain()
                wait_clock.add_sem_waits(
                    d.ins, ScopedClock({None: VectorClock(vec)})
                )
        else:
            d = self.nc.sync.drain()
            wait_clock.add_sem_waits(d.ins, ScopedClock({None: gc}))
        self.nc.all_engine_barrier()
        assert self.sems is not None
        popped = self.nc._tile_sem_poison_stack.pop()
        assert popped is self._sem_poison
        self.nc.clear_and_free_semaphores(list(self.sems.allocated().values()))
        self.nc.all_engine_barrier()

    tile_mod.TileContext._drain_and_barrier = _split_drain_and_barrier
    tile_mod.TileContext._drain_split_installed = True


def _install_usersync_prep():
    """Route gen_mode==1 SWDGE gather preps onto their ENGINE proc
    (user-synced protocol) instead of a DMASW lane: Tile's DMASW-lane
    path for preps emits a pre-bumped doorbell + mismatched completion
    sem and deadlocks/races on this build. With the engine tick also
    registered in prep_eng_ticks, pass 2 gates trigger_dma on desc-gen
    completion; data completion is via the caller's sem= semaphore and
    explicit _wait_ge on consumer instructions."""
    import concourse.tile_sem_assignment as tsa
    import concourse.mybir as mybir

    if getattr(tsa.TileClockTick, "_usersync_prep_installed", False):
        return
    orig = tsa.TileClockTick._assign_tick

    def patched(self, inst):
        if getattr(inst, "gen_mode", 0) == 1 and isinstance(
            inst, (mybir.InstDMAGatherAnt, mybir.InstDMAScatterAddAnt)
        ):
            eng_proc_idx = (
                tsa.ENGINE_SEQUENCER_TO_IDX
                if inst.is_sequencer_only()
                else tsa.ENGINE_TO_IDX
            )[inst.engine]
            tick = self.global_clock.advance(eng_proc_idx)
            inst.bass_scheduled_tick = tick
            inst.bass_scheduled_proc = eng_proc_idx
            inst.bass_scheduled_scope = self.scope_name
            self._proc_insts[self.root_scope_name][eng_proc_idx].append(inst)
            self.tc.prep_eng_ticks[inst.name] = (eng_proc_idx, tick)
            self._prep_eng_names[self.root_scope_name].append(inst.name)
            return
        return orig(self, inst)

    tsa.TileClockTick._assign_tick = patched
    tsa.TileClockTick._usersync_prep_installed = True


# ---------------------------------------------------------------------------
def _wrap_idx(seq):
    """int16 index layout for dma_gather: position i -> [i%16, i//16],
    replicated across the 8 groups of 16 partitions."""
    a = np.asarray(seq, np.int16)
    L = len(a)
    assert L % 16 == 0
    a16 = a.reshape(L // 16, 16).T  # [16, L/16]
    return np.ascontiguousarray(np.tile(a16, (8, 1)))  # [128, L/16]


def _prepare(inputs, D, P, CH, NCH):
    """Host-side index preprocessing + weight/emb staging (layout only)."""
    PP = CH * NCH
    node_indices = np.asarray(inputs["node_indices"])
    parent_indices = np.asarray(inputs["parent_indices"])
    k = (parent_indices > 0).sum(-1)  # [D, P]
    recent = np.zeros((D, P), bool)
    for d in range(1, D):
        recent[d] = (parent_indices[d] >= 2 + (d - 1) * P).any(-1)

    remap = np.zeros(2 + D * P, np.int64)
    remap[1] = 1
    perms = []
    for d in range(D):
        # old-parent nodes first, then by parent count desc (prefix trick)
        perm = np.lexsort((-k[d], recent[d]))
        perms.append(perm)
        remap[2 + d * P + perm] = 2 + d * PP + np.arange(P)

    chunk_meta = []  # [d][c] -> dict(blens, adds, bound, w16)
    pidx_chunks = {}
    MAXW16 = 0
    MAXB = 0
    for d in range(D):
        perm = perms[d]
        row = []
        for c in range(NCH):
            lo = c * CH
            hi = min(lo + CH, P)
            nodes = perm[lo:hi]
            nreal = len(nodes)
            kc = k[d][nodes]
            rc = recent[d][nodes]
            og = int((~rc).sum())
            if d == 0:
                bound = 2
            elif og == nreal:
                bound = 2 + (d - 1) * PP
            else:
                bound = 2 + d * PP

            # segments: (col_start, idx_array); slot0 first (becomes pv)
            segs = [(0, np.pad(remap[parent_indices[d, nodes, 0]],
                               (0, CH - nreal)))]
            for j in range(1, MP):
                mo = int((kc[:og] > j).sum())
                if mo:
                    segs.append((0, remap[parent_indices[d, nodes[:mo], j]]))
                mr = int((kc[og:] > j).sum())
                if mr:
                    segs.append(
                        (og, remap[parent_indices[d, nodes[og:og + mr], j]]))

            # pack segments into <=BCAP bundles; record add ops
            blens, adds = [], []
            wrapped = []
            cur, cur_len = [], 0

            def close():
                nonlocal cur, cur_len
                if not cur_len:
                    return
                idx = np.concatenate(cur)
                L = (len(idx) + 127) // 128 * 128
                idx = np.pad(idx, (0, L - len(idx)))
                wrapped.append(_wrap_idx(idx))
                blens.append(L)
                cur, cur_len = [], 0

            for si, (cs, arr) in enumerate(segs):
                L = len(arr)
                if cur_len + L > BCAP and cur_len > 0:
                    close()
                if si > 0:
                    adds.append((len(blens), cur_len, cs, L))
                cur.append(arr)
                cur_len += L
            close()

            w16 = sum(L // 16 for L in blens)
            pidx_chunks[(d, c)] = np.concatenate(wrapped, axis=1)
            MAXW16 = max(MAXW16, w16)
            MAXB = max(MAXB, len(blens))
            row.append({"blens": blens, "adds": adds, "bound": bound,
                        "w16": w16})
        chunk_meta.append(row)

    pidx_np = np.zeros((D, NCH, 128, MAXW16), np.int16)
    for (d, c), w in pidx_chunks.items():
        pidx_np[d, c, :, : w.shape[1]] = w

    emb = np.asarray(inputs["emb_table"], np.float32)
    Wout = np.asarray(inputs["Wout"], np.float32)
    W1 = np.asarray(inputs["W1"], np.float32)
    W2 = np.asarray(inputs["W2"], np.float32)

    # per-chunk feature-major emb rows + Wout rows: [D, NCH, h, 2, CH]
    newt = np.zeros((D, NCH, H, 2, CH), np.float32)
    for d in range(D):
        for c in range(NCH):
            lo = c * CH
            hi = min(lo + CH, P)
            nodes = perms[d][lo:hi]
            newt[d, c, :, 0, : hi - lo] = emb[node_indices[d][nodes]].T
            newt[d, c, :, 1, : hi - lo] = Wout[1 + d * P + nodes].T

    wcat = np.zeros((128, 5, 128), np.float32)
    wcat[:, 0] = W1[:, :H].T
    wcat[:, 1] = W1[:, H:].T
    wcat[:, 2] = W2.T
    wcat[:, 3] = np.eye(128)
    wcat[:, 4, 0] = 1.0  # ones column for the partition-reduce matmul

    prep = {
        "meta": chunk_meta,
        "perms": perms,
        "maxw16": MAXW16,
        "pidx": pidx_np,
        "newt": np.ascontiguousarray(newt.astype(BF16)),
        "wcat": np.ascontiguousarray(wcat.astype(BF16)),
        "b1": np.asarray(inputs["b1"], np.float32).reshape(128, 1),
        "b2": np.asarray(inputs["b2"], np.float32).reshape(128, 1),
    }
    return prep


def _build(prep, D, P, CH, NCH):
    """Trace the Bass/Tile kernel. Returns a finalized Bacc."""
    import os
    PREP = os.environ.get("KPREP", "1") == "1"
    PREPN = int(os.environ.get("KPREPN", "99999"))
    _install_tilefix()
    if PREP:
        _install_usersync_prep()
    from contextlib import ExitStack

    import concourse.bacc as bacc
    import concourse.mybir as mybir
    from concourse.tile import TileContext

    PP = CH * NCH
    TOK = 2 + D * PP
    ROW = BL * H  # nv row elems (bf16)
    KB = CH // 128
    f32 = mybir.dt.float32
    bf16 = mybir.dt.bfloat16
    i16 = mybir.dt.int16
    AF = mybir.ActivationFunctionType

    nc = bacc.Bacc("TRN2", target_bir_lowering=False, debug=False)

    nv = nc.dram_tensor("nv", [TOK, ROW], bf16, kind="Internal")
    nvinit = nc.dram_tensor("nvinit", [2, ROW], bf16, kind="ExternalInput")
    pidx_in = nc.dram_tensor(
        "pidx", list(prep["pidx"].shape), i16, kind="ExternalInput"
    )
    newt_in = nc.dram_tensor(
        "newt", [D, NCH, 128, 2, CH], bf16, kind="ExternalInput"
    )
    wcat_in = nc.dram_tensor("wcat", [128, 5, 128], bf16, kind="ExternalInput")
    b1_in = nc.dram_tensor("b1c", [128, 1], f32, kind="ExternalInput")
    b2_in = nc.dram_tensor("b2c", [128, 1], f32, kind="ExternalInput")
    outd = nc.dram_tensor("outd", [D, NCH, 4, 2, CH], f32,
                          kind="ExternalOutput")

    meta = prep["meta"]

    with TileContext(nc) as tc, ExitStack() as ctx:
        const = ctx.enter_context(tc.tile_pool(name="const", bufs=1))
        pidx_pool = ctx.enter_context(tc.tile_pool(name="pidx", bufs=3))
        stag_pool = ctx.enter_context(tc.tile_pool(name="stag", bufs=3))
        newt_pool = ctx.enter_context(tc.tile_pool(name="newt", bufs=3))
        h1_pool = ctx.enter_context(tc.tile_pool(name="h1", bufs=3))
        nvn_pool = ctx.enter_context(tc.tile_pool(name="nvn", bufs=8))
        m_pool = ctx.enter_context(tc.tile_pool(name="m", bufs=2))
        nvrm_pool = ctx.enter_context(tc.tile_pool(name="nvrm", bufs=3))
        outsb_pool = ctx.enter_context(tc.tile_pool(name="outsb", bufs=2))
        psmm = ctx.enter_context(tc.tile_pool(name="psmm", bufs=2, space="PSUM"))
        psm2 = ctx.enter_context(tc.tile_pool(name="psm2", bufs=2, space="PSUM"))
        pstp = ctx.enter_context(tc.tile_pool(name="pstp", bufs=2, space="PSUM"))
        pso = ctx.enter_context(tc.tile_pool(name="pso", bufs=1, space="PSUM"))

        # rotation depth must be >= stag pool bufs+1 so same-sem chunks
        # can never have gathers in flight concurrently
        dma_sems = [nc.alloc_semaphore(f"swdma{i}") for i in range(4)]
        gath_cnt = [0, 0, 0, 0]

        wcat = const.tile([128, 5, 128], bf16)
        nc.sync.dma_start(out=wcat[:], in_=wcat_in[:, :, :])
        w1at = wcat[:, 0, :]
        w1bt = wcat[:, 1, :]
        w2t = wcat[:, 2, :]
        identb = wcat[:, 3, :]
        ones_col = wcat[:, 4, 0:1]
        b1 = const.tile([128, 1], f32)
        nc.sync.dma_start(out=b1[:], in_=b1_in[:, :])
        b2 = const.tile([128, 1], f32)
        nc.sync.dma_start(out=b2[:], in_=b2_in[:, :])

        # init nv rows 0..1 (zero pad row + root = per-b embedding)
        import bass_rust as _br
        _DI = _br.DependencyInfo(sync=True, no_sync=False)
        nv_writes = {}  # depth -> [dma inst names]
        nvi = const.tile([2, ROW], bf16)
        nc.sync.dma_start(out=nvi[:], in_=nvinit[:, :])
        winit = nc.sync.dma_start(out=nv[0:2, :], in_=nvi[:])
        nv_writes[-1] = [winit.ins.name]

        nchunk = 0
        for d in range(D):
            for c in range(NCH):
                cprep = PREP and nchunk < PREPN
                par = nchunk % 4
                dma_sem = dma_sems[par]
                nchunk += 1
                md = meta[d][c]
                blens, adds, bound = md["blens"], md["adds"], md["bound"]

                pidx_sb = pidx_pool.tile([128, md["w16"]], i16)
                nc.sync.dma_start(
                    out=pidx_sb[:], in_=pidx_in[d, c, :, : md["w16"]]
                )
                newt_sb = newt_pool.tile([128, 2, CH], bf16)
                nc.sync.dma_start(out=newt_sb[:], in_=newt_in[d, c])

                # source-writer sync deps go on the first PREP (the
                # trigger cannot carry >1 sem wait on this build)
                if d == 0:
                    wdeps = nv_writes[-1]
                elif bound == 2 + (d - 1) * PP:
                    wdeps = nv_writes.get(d - 2, nv_writes[-1])
                else:
                    wdeps = nv_writes[d - 1]
                stags = []
                off16 = 0
                for bi, L in enumerate(blens):
                    stag = stag_pool.tile([128, BL, L], bf16, tag=f"stag{bi}")
                    if cprep:
                        g = nc.gpsimd.dma_gather(
                            stag[:], nv[0:bound, :],
                            pidx_sb[:, off16 : off16 + L // 16],
                            num_idxs=L, num_idxs_reg=L,
                            elem_size=ROW, transpose=True,
                            prepare_only=True, sem=dma_sem,
                        )
                        for wn in wdeps:
                            g.ins.add_dependency(wn, _DI)
                        nc.gpsimd.trigger_dma(count=None)
                    else:
                        nc.gpsimd.dma_gather(
                            stag[:], nv[0:bound, :],
                            pidx_sb[:, off16 : off16 + L // 16],
                            num_idxs=L, num_idxs_reg=L,
                            elem_size=ROW, transpose=True,
                        )
                    off16 += L // 16
                    stags.append(stag)
                if cprep:
                    gath_cnt[par] += len(blens)
                gtarget = 16 * gath_cnt[par]

                # ---- parent-slot reduction in place into slot-0 segment
                pv = stags[0]
                for (bi, off, cs, L) in adds:
                    a = nc.vector.tensor_add(
                        pv[:, :, cs : cs + L],
                        pv[:, :, cs : cs + L],
                        stags[bi][:, :, off : off + L],
                    )
                    if cprep:
                        a._wait_ge(dma_sem, gtarget)

                # ---- MLP (bf16) over col pairs (2 b's x CH = 512 cols)
                ne_b = newt_sb[:, 0:1, :]
                nvns = []
                for bp in range(BL // 2):
                    pv2 = pv[:, 2 * bp : 2 * bp + 2, 0:CH]
                    h1p = psmm.tile([128, 2, CH], f32, tag="h1p")
                    mm = nc.tensor.matmul(
                        h1p[:], lhsT=w1at, rhs=pv2, start=True, stop=False
                    )
                    if cprep:
                        mm._wait_ge(dma_sem, gtarget)
                    nc.tensor.matmul(
                        h1p[:], lhsT=w1bt,
                        rhs=ne_b.to_broadcast([128, 2, CH]),
                        start=False, stop=True,
                    )
                    h1 = h1_pool.tile([128, 2, CH], bf16)
                    nc.scalar.activation(h1[:], h1p[:], AF.Relu, bias=b1[:])
                    h2p = psm2.tile([128, 2, CH], f32, tag="h2p")
                    nc.tensor.matmul(
                        h2p[:], lhsT=w2t, rhs=h1[:], start=True, stop=True
                    )
                    nvt = nvn_pool.tile([128, 2, CH], bf16)
                    nc.scalar.activation(nvt[:], h2p[:], AF.Identity,
                                         bias=b2[:])
                    r = nc.vector.tensor_add(nvt[:], nvt[:], pv2)  # residual
                    if cprep:
                        r._wait_ge(dma_sem, gtarget)
                    nvns.append(nvt)

                # ---- fused out-projection: m = nvn * woutT; ones-reduce
                wT = newt_sb[:, 1:2, :]
                m_all = m_pool.tile([128, BL, CH], bf16)
                pso_ts = []
                for t in range(2):
                    pso_t = pso.tile([128, 2, CH], f32, tag=f"pso{t}")
                    pso_ts.append(pso_t)
                for bp in range(BL // 2):
                    nc.vector.tensor_mul(
                        m_all[:, 2 * bp : 2 * bp + 2, :],
                        nvns[bp][:],
                        wT.to_broadcast([128, 2, CH]),
                    )
                    po = 32 * (bp % 2)
                    nc.tensor.matmul(
                        pso_ts[bp // 2][po : po + 1, :, :],
                        lhsT=ones_col,
                        rhs=m_all[:, 2 * bp : 2 * bp + 2, :],
                        start=True, stop=True,
                    )
                outsb = outsb_pool.tile([128, 2, 2, CH], f32)
                for t in range(2):
                    nc.scalar.copy(out=outsb[0:64, t, :, :],
                                   in_=pso_ts[t][0:64, :, :])
                    nc.sync.dma_start(out=outd[d, c, 2 * t : 2 * t + 2],
                                      in_=outsb[0:64:32, t, :, :])

                # ---- transpose back (bf16) and write token rows
                for kb in range(KB):
                    tp = pstp.tile([128, BL, 128], bf16, tag="tp")
                    for b in range(BL):
                        nc.tensor.transpose(
                            tp[:, b, :],
                            nvns[b // 2][:, b % 2, kb * 128 : (kb + 1) * 128],
                            identb,
                        )
                    nvrm = nvrm_pool.tile([128, BL, 128], bf16)
                    nc.vector.tensor_copy(out=nvrm[:], in_=tp[:])
                    tokbase = 2 + d * PP + c * CH + kb * 128
                    wnv = nc.sync.dma_start(
                        out=nv[tokbase : tokbase + 128, :],
                        in_=nvrm[:].rearrange("p b h -> p (b h)"),
                    )
                    nv_writes.setdefault(d, []).append(wnv.ins.name)

    nc.finalize()
    return nc


def _run_cores(nc, prep, embedding, n_cores):
    from concourse import bass_utils

    in_maps = []
    base = {
        "pidx": prep["pidx"],
        "newt": prep["newt"],
        "wcat": prep["wcat"],
        "b1c": prep["b1"],
        "b2c": prep["b2"],
    }
    for core in range(n_cores):
        eb = embedding[core * BL : (core + 1) * BL]  # [BL, H]
        nvinit = np.zeros((2, BL * H), np.float32)
        nvinit[1] = eb.reshape(-1)
        m = dict(base)
        m["nvinit"] = np.ascontiguousarray(nvinit.astype(BF16))
        in_maps.append(m)
    res = bass_utils.run_bass_kernel_spmd(
        nc, in_maps, core_ids=list(range(n_cores))
    )
    global LAST_RESULTS
    LAST_RESULTS = res
    return res


def _assemble(results, prep, inputs, D, P, CH, NCH, n_cores):
    embedding = np.asarray(inputs["embedding"], np.float32)
    Wout = np.asarray(inputs["Wout"], np.float32)
    bout = np.asarray(inputs["bout"], np.float32)
    NTOT = 1 + D * P

    out = np.empty((embedding.shape[0], NTOT), np.float32)
    out[:, 0] = embedding @ Wout[0] + bout[0]
    for core in range(n_cores):
        v = results[core]["outd"]  # [D, NCH, 4, 2, CH]
        for d in range(D):
            sg = np.concatenate(
                [v[d, c].reshape(BL, CH) for c in range(NCH)], axis=1
            )  # [BL, PP], col = sorted position
            cols = 1 + d * P + prep["perms"][d]
            out[core * BL : (core + 1) * BL, cols] = sg[:, :P]
    out[:, 1:] += bout[None, 1:]
    return out


def kernel(**inputs):
    D, P, CH, NCH = D_FULL, P_FULL, 256, 4
    prep = _prepare(inputs, D, P, CH, NCH)
    nc = _build(prep, D, P, CH, NCH)
    res = _run_cores(nc, prep, np.asarray(inputs["embedding"], np.float32), NCORES)
    return _assemble(res.results, prep, inputs, D, P, CH, NCH, NCORES)



# revision 13
# speedup vs baseline: 1.2323x; 1.2323x over previous
"""Trainium2 Bass kernel for nn_DAGModel (gnn_message_passing).

Strategy (data-parallel over batch, 8 b's per core, node-major gathers):
- node_vecs live in DRAM as a bf16 table `nv[token, b8*h128]` (2KB rows).
- Parent gathers use GPSIMD dma_gather(transpose=False): gathered row i
  lands whole in partition i%128 / free slot i//128, so each row is ONE
  tx + ONE rx descriptor (vs 8 rx descs/row for transpose=True). Indices
  are laid out (slot, node): slot s of node p sits at position s*128+p,
  so slot blocks land as [node_partition, slot, b*h] tiles and the
  parent-slot sum is a plain free-dim tensor_add. Pad slots use index 0
  (the zero row).
- Descriptor generation (prepare_only) carries NO data dependency; the
  data dependency (prior depths' nv row writes) sits on a gpsimd
  wait_ge(nvw) ahead of the trigger_dma, where nvw counts nv writeback
  DMA completions. Groups whose parents all come from depths <= d-2 are
  ordered first and trigger with a lower nvw target, overlapping the
  previous depth's tail compute. Gathers rotate across 4 SWDGE queues.
- Depth 0 does no gathers at all: every parent is the root, so
  pv = k[node] * root (one DVE tensor_scalar_mul per 128-node group
  against a host-precomputed broadcast root tile).
- pv is PE-transposed per b (8x [128x128]) to h-major, the 2-layer MLP
  runs in bf16 on the PE (f32 PSUM accumulate) over 1024-column tiles,
  ReLU+b1 / +b2 ride the Scalar engine, then h2 is PE-transposed back to
  node-major and the residual add (pv + h2) fuses with the PSUM->SBUF
  move on the DVE, yielding the row-major writeback tile directly.
- Output projection is node-major: out[n,b] = sum_h nvrm[n,b,h]*Wout[n,h]
  via DVE tensor_mul + tensor_reduce(axis=X) - no PE or Scalar involved.
"""

import os

import numpy as np
import ml_dtypes

BF16 = ml_dtypes.bfloat16

# Full-problem dims (hardcoded per contract).
B, H, E = 64, 128, 128
D, P, MP = 20, 1000, 8
D = int(os.environ.get("KD", D))  # debug: truncate depth loop
NCORES, BL = 8, 8
G, GN = 8, 128          # groups per depth, nodes per group
PP = G * GN             # padded nodes per depth (1024)
TOK = 2 + D * PP        # nv rows
ROW = BL * H            # nv row elems (bf16)
NQ = int(os.environ.get("KNQ", "3"))                  # SWDGE queues
LAST_RESULTS = None


# ---------------------------------------------------------------------------
# workaround: this walrus build rejects >1 sync-wait on a CTRL (Drain) inst.
def _install_tilefix():
    import concourse.tile as tile_mod
    from concourse.vector_clock import ScopedClock, VectorClock

    if getattr(tile_mod.TileContext, "_drain_split_installed", False):
        return

    def _split_drain_and_barrier(self, tick_clock, wait_clock):
        gc = tick_clock.global_clock
        ticks = list(gc)
        nz = [(i, t) for i, t in enumerate(ticks) if t > 0]
        if nz:
            for i, t in nz:
                vec = [0] * len(ticks)
                vec[i] = t
                d = self.nc.sync.drain()
                wait_clock.add_sem_waits(
                    d.ins, ScopedClock({None: VectorClock(vec)})
                )
        else:
            d = self.nc.sync.drain()
            wait_clock.add_sem_waits(d.ins, ScopedClock({None: gc}))
        self.nc.all_engine_barrier()
        assert self.sems is not None
        popped = self.nc._tile_sem_poison_stack.pop()
        assert popped is self._sem_poison
        self.nc.clear_and_free_semaphores(list(self.sems.allocated().values()))
        self.nc.all_engine_barrier()

    tile_mod.TileContext._drain_and_barrier = _split_drain_and_barrier
    tile_mod.TileContext._drain_split_installed = True


def _install_usersync_prep():
    """Route gen_mode==1 SWDGE gather preps onto their ENGINE proc
    (user-synced protocol) instead of a DMASW lane: Tile's DMASW-lane
    path for preps emits a pre-bumped doorbell + mismatched completion
    sem and deadlocks/races on this build. With the engine tick also
    registered in prep_eng_ticks, pass 2 gates trigger_dma on desc-gen
    completion; data completion is via the caller's sem= semaphore and
    explicit _wait_ge on consumer instructions."""
    import concourse.tile_sem_assignment as tsa
    import concourse.mybir as mybir

    if getattr(tsa.TileClockTick, "_usersync_prep_installed", False):
        return
    orig = tsa.TileClockTick._assign_tick

    def patched(self, inst):
        if getattr(inst, "gen_mode", 0) == 1 and isinstance(
            inst, (mybir.InstDMAGatherAnt, mybir.InstDMAScatterAddAnt)
        ):
            eng_proc_idx = (
                tsa.ENGINE_SEQUENCER_TO_IDX
                if inst.is_sequencer_only()
                else tsa.ENGINE_TO_IDX
            )[inst.engine]
            tick = self.global_clock.advance(eng_proc_idx)
            inst.bass_scheduled_tick = tick
            inst.bass_scheduled_proc = eng_proc_idx
            inst.bass_scheduled_scope = self.scope_name
            self._proc_insts[self.root_scope_name][eng_proc_idx].append(inst)
            self.tc.prep_eng_ticks[inst.name] = (eng_proc_idx, tick)
            self._prep_eng_names[self.root_scope_name].append(inst.name)
            return
        return orig(self, inst)

    tsa.TileClockTick._assign_tick = patched
    tsa.TileClockTick._usersync_prep_installed = True


# ---------------------------------------------------------------------------
def _wrap_idx(seq):
    """int16 index layout for dma_gather: position i -> [i%16, i//16],
    replicated across the 8 groups of 16 partitions."""
    a = np.asarray(seq, np.int16)
    L = len(a)
    assert L % 16 == 0
    a16 = a.reshape(L // 16, 16).T  # [16, L/16]
    return np.ascontiguousarray(np.tile(a16, (8, 1)))  # [128, L/16]


def _prepare(inputs):
    """Host-side index preprocessing + weight/emb staging (layout only)."""
    node_indices = np.asarray(inputs["node_indices"])
    parent_indices = np.asarray(inputs["parent_indices"])
    emb = np.asarray(inputs["emb_table"], np.float32)
    Wout = np.asarray(inputs["Wout"], np.float32)
    W1 = np.asarray(inputs["W1"], np.float32)
    W2 = np.asarray(inputs["W2"], np.float32)

    k = (parent_indices > 0).sum(-1)  # [D, P]
    recent = np.zeros((D, P), bool)
    for d in range(1, D):
        recent[d] = (parent_indices[d] >= 2 + (d - 1) * P).any(-1)

    # old-parent nodes first, then by parent count desc
    perms = [np.lexsort((-k[d], recent[d])) for d in range(D)]
    remap = np.zeros(2 + D * P, np.int64)
    remap[1] = 1
    for d in range(D):
        remap[2 + d * P + perms[d]] = 2 + d * PP + np.arange(P)

    pidx_np = np.zeros((D, 128, G, 8 * MP), np.int16)
    newt_np = np.zeros((D, 128, G, 2, GN), np.float32)
    kvec_np = np.zeros((128, G), np.float32)
    meta = []  # [d] -> list of dicts per group
    for d in range(D):
        row = []
        for g in range(G):
            nodes = perms[d][g * GN : (g + 1) * GN]
            nreal = len(nodes)
            kg = k[d][nodes]
            kmax = max(1, int(kg.max()) if nreal else 1)
            newt_np[d, :, g, 0, :nreal] = emb[node_indices[d][nodes]].T
            newt_np[d, :nreal, g, 1, :] = Wout[1 + d * P + nodes]
            if d == 0:
                kvec_np[:nreal, g] = kg
                row.append({"kmax": kmax, "old": True})
                continue
            idx = np.zeros((kmax, GN), np.int64)
            for s in range(kmax):
                sel = kg > s
                idx[s, : nreal][sel] = remap[parent_indices[d, nodes[sel], s]]
            old = bool(idx.max() < 2 + (d - 1) * PP)
            pidx_np[d, :, g, : 8 * kmax] = _wrap_idx(idx.reshape(-1))
            row.append({"kmax": kmax, "old": old})
        meta.append(row)

    wcat = np.zeros((128, 4, 128), np.float32)
    wcat[:, 0] = W1[:, :H].T
    wcat[:, 1] = W1[:, H:].T
    wcat[:, 2] = W2.T
    wcat[:, 3] = np.eye(128)

    return {
        "meta": meta,
        "perms": perms,
        "pidx": pidx_np,
        "newt": np.ascontiguousarray(newt_np.astype(BF16)),
        "kvec": np.ascontiguousarray(kvec_np),
        "wcat": np.ascontiguousarray(wcat.astype(BF16)),
        "b1": np.asarray(inputs["b1"], np.float32).reshape(128, 1),
        "b2": np.asarray(inputs["b2"], np.float32).reshape(128, 1),
    }


def _build(prep):
    """Trace the Bass/Tile kernel. Returns a finalized Bacc."""
    _install_tilefix()
    _install_usersync_prep()
    from contextlib import ExitStack

    import concourse.bacc as bacc
    import concourse.mybir as mybir
    from concourse.tile import TileContext

    f32 = mybir.dt.float32
    bf16 = mybir.dt.bfloat16
    i16 = mybir.dt.int16
    AF = mybir.ActivationFunctionType
    AX = mybir.AxisListType
    ALU = mybir.AluOpType

    nc = bacc.Bacc(
        "TRN2", target_bir_lowering=False, debug=False, num_swdge_queues=NQ
    )

    nv = nc.dram_tensor("nv", [TOK, ROW], bf16, kind="Internal")
    nvinit = nc.dram_tensor("nvinit", [2, ROW], bf16, kind="ExternalInput")
    rootnm_in = nc.dram_tensor("rootnm", [128, ROW], bf16, kind="ExternalInput")
    kvec_in = nc.dram_tensor("kvecc", [128, G], f32, kind="ExternalInput")
    pidx_in = nc.dram_tensor(
        "pidx", [D, 128, G, 8 * MP], i16, kind="ExternalInput"
    )
    newt_in = nc.dram_tensor(
        "newt", [D, 128, G, 2, GN], bf16, kind="ExternalInput"
    )
    wcat_in = nc.dram_tensor("wcat", [128, 4, 128], bf16, kind="ExternalInput")
    b1_in = nc.dram_tensor("b1c", [128, 1], f32, kind="ExternalInput")
    b2_in = nc.dram_tensor("b2c", [128, 1], f32, kind="ExternalInput")
    outd = nc.dram_tensor("outd", [D, 128, G, BL], f32, kind="ExternalOutput")

    meta = prep["meta"]

    with TileContext(nc) as tc, ExitStack() as ctx:
        const = ctx.enter_context(tc.tile_pool(name="const", bufs=1))
        pidx_pool = ctx.enter_context(tc.tile_pool(name="pidx", bufs=3))
        newt_pool = ctx.enter_context(tc.tile_pool(name="newt", bufs=3))
        stag_pool = ctx.enter_context(tc.tile_pool(name="stag", bufs=6))
        pvt_pool = ctx.enter_context(tc.tile_pool(name="pvt", bufs=2))
        h1_pool = ctx.enter_context(tc.tile_pool(name="h1", bufs=2))
        h2_pool = ctx.enter_context(tc.tile_pool(name="h2", bufs=2))
        nvrm_pool = ctx.enter_context(tc.tile_pool(name="nvrm", bufs=3))
        m_pool = ctx.enter_context(tc.tile_pool(name="m", bufs=2))
        out_pool = ctx.enter_context(tc.tile_pool(name="outp", bufs=2))
        psA = ctx.enter_context(tc.tile_pool(name="psA", bufs=2, space="PSUM"))
        psB = ctx.enter_context(tc.tile_pool(name="psB", bufs=2, space="PSUM"))
        psC = ctx.enter_context(tc.tile_pool(name="psC", bufs=2, space="PSUM"))
        psT = ctx.enter_context(tc.tile_pool(name="psT", bufs=2, space="PSUM"))

        qsems = [nc.alloc_semaphore(f"swdma{i}") for i in range(NQ)]
        gdum = nc.alloc_semaphore("gdum")
        import bass_rust as _br
        _DI = _br.DependencyInfo(sync=True, no_sync=False)
        last_write = {}  # depth -> last nv-write inst name (-1 = init)

        wcat = const.tile([128, 4, 128], bf16)
        nc.sync.dma_start(out=wcat[:], in_=wcat_in[:, :, :])
        w1at = wcat[:, 0, :]
        w1bt = wcat[:, 1, :]
        w2t = wcat[:, 2, :]
        identb = wcat[:, 3, :]
        b1 = const.tile([128, 1], f32)
        nc.sync.dma_start(out=b1[:], in_=b1_in[:, :])
        b2 = const.tile([128, 1], f32)
        nc.sync.dma_start(out=b2[:], in_=b2_in[:, :])
        kvec = const.tile([128, G], f32)
        nc.sync.dma_start(out=kvec[:], in_=kvec_in[:, :])
        rootnm = const.tile([128, ROW], bf16)
        nc.sync.dma_start(out=rootnm[:], in_=rootnm_in[:, :])

        # init nv rows 0..1 (zero pad row + root = per-b embedding)
        nvi = const.tile([2, ROW], bf16)
        nc.sync.dma_start(out=nvi[:], in_=nvinit[:, :])
        winit = nc.sync.dma_start(out=nv[0:2, :], in_=nvi[:])
        last_write[-1] = winit.ins.name

        gcnt = [0] * NQ  # gathers ever assigned per queue
        gq = 0           # round-robin counter

        for d in range(D):
            pidx_sb = None
            if d > 0:
                pidx_sb = pidx_pool.tile([128, G, 8 * MP], i16)
                nc.sync.dma_start(out=pidx_sb[:], in_=pidx_in[d])
            newt_sb = newt_pool.tile([128, G, 2, GN], bf16)
            nc.sync.dma_start(out=newt_sb[:], in_=newt_in[d])
            outacc = out_pool.tile([128, G, BL], f32)

            groups = meta[d]
            old_gs = [g for g in range(G) if groups[g]["old"]]
            rec_gs = [g for g in range(G) if not groups[g]["old"]]
            phases = []
            if d == 0:
                phases.append((old_gs + rec_gs, None, None))
            else:
                if old_gs:
                    phases.append((old_gs, 2 + (d - 1) * PP, max(d - 2, -1)))
                if rec_gs:
                    phases.append((rec_gs, 2 + d * PP, d - 1))

            for gs, bound, wdep in phases:
                ginfo = {}
                if d > 0:
                    # desc-gen in sub-phases of <= NQ groups (bounded by the
                    # stag pool depth so a prep's WAR wait can never precede
                    # its victim's trigger), each: preps -> data wait ->
                    # per-queue triggers. Preps carry no data deps.
                    for sub in range(0, len(gs), NQ):
                        qs_used = []
                        for g in gs[sub : sub + NQ]:
                            kmax = groups[g]["kmax"]
                            stag = stag_pool.tile([128, MP, ROW], bf16)
                            q = gq % NQ
                            gq += 1
                            nc.gpsimd.dma_gather(
                                stag[:, :kmax, :],
                                nv[0:bound, :],
                                pidx_sb[:, g, : 8 * kmax],
                                num_idxs=GN * kmax,
                                num_idxs_reg=GN * kmax,
                                elem_size=ROW,
                                transpose=False,
                                prepare_only=True,
                                sem=qsems[q],
                                queue_num=q,
                            )
                            gcnt[q] += 1
                            ginfo[g] = (stag, q, gcnt[q], kmax)
                            if q not in qs_used:
                                qs_used.append(q)
                        # data gate: Pool waits for the last nv write the
                        # phase's sources depend on, then fires the triggers
                        car = nc.gpsimd.sem_inc(gdum, 1)
                        car.ins.add_dependency(last_write[wdep], _DI)
                        for q in qs_used:
                            nc.gpsimd.trigger_dma(count=None, queue_num=q)

                for g in gs:
                    kmax = groups[g]["kmax"]
                    if d == 0:
                        stag = stag_pool.tile([128, MP, ROW], bf16)
                        pv = stag[:, 0, :]
                        nc.vector.tensor_scalar_mul(
                            pv, rootnm[:], kvec[:, g : g + 1]
                        )
                        need_wait = None
                    else:
                        stag, q, cnt, kmax = ginfo[g]
                        pv = stag[:, 0, :]
                        need_wait = (qsems[q], 16 * cnt)
                        for s in range(1, kmax):
                            a = nc.vector.tensor_add(pv, pv, stag[:, s, :])
                            if need_wait is not None:
                                a._wait_ge(*need_wait)
                                need_wait = None

                    # ---- fwd transposes: pv [node, b*h] -> pvT [h, b, node]
                    pvt_ps = psA.tile([128, BL, 128], bf16, tag="pvt")
                    for b in range(BL):
                        t = nc.tensor.transpose(
                            pvt_ps[:, b, :],
                            pv[:, b * 128 : (b + 1) * 128],
                            identb,
                        )
                        if need_wait is not None:
                            t._wait_ge(*need_wait)
                            need_wait = None
                    pvt_sb = pvt_pool.tile([128, BL, 128], bf16)
                    nc.vector.tensor_copy(out=pvt_sb[:], in_=pvt_ps[:])

                    # ---- MLP in h-major (512-column halves: PSUM bank limit)
                    ne_b = newt_sb[:, g, 0:1, :]
                    h1 = h1_pool.tile([128, BL, 128], bf16)
                    h2sb = h2_pool.tile([128, BL, 128], bf16)
                    HB = BL // 2
                    for hf in range(2):
                        sl = slice(HB * hf, HB * (hf + 1))
                        h1p = psB.tile([128, HB, 128], f32, tag="h1p")
                        nc.tensor.matmul(
                            h1p[:], lhsT=w1at, rhs=pvt_sb[:, sl, :],
                            start=True, stop=False,
                        )
                        nc.tensor.matmul(
                            h1p[:], lhsT=w1bt,
                            rhs=ne_b.to_broadcast([128, HB, GN]),
                            start=False, stop=True,
                        )
                        nc.scalar.activation(
                            h1[:, sl, :], h1p[:], AF.Relu, bias=b1[:]
                        )
                        h2p = psC.tile([128, HB, 128], f32, tag="h2p")
                        nc.tensor.matmul(
                            h2p[:], lhsT=w2t, rhs=h1[:, sl, :],
                            start=True, stop=True,
                        )
                        nc.scalar.activation(
                            h2sb[:, sl, :], h2p[:], AF.Identity, bias=b2[:]
                        )

                    # ---- back transposes + residual fused with PSUM->SBUF
                    tp = psT.tile([128, BL, 128], bf16, tag="tp")
                    for b in range(BL):
                        nc.tensor.transpose(tp[:, b, :], h2sb[:, b, :], identb)
                    nvrm = nvrm_pool.tile([128, BL, 128], bf16)
                    nc.vector.tensor_add(
                        nvrm[:].rearrange("p b h -> p (b h)"),
                        tp[:].rearrange("p b h -> p (b h)"),
                        pv,
                    )
                    tokbase = 2 + d * PP + g * GN
                    wnv = nc.sync.dma_start(
                        out=nv[tokbase : tokbase + GN, :],
                        in_=nvrm[:].rearrange("p b h -> p (b h)"),
                    )
                    last_write[d] = wnv.ins.name

                    # ---- out-projection, node-major on DVE
                    wo = newt_sb[:, g, 1:2, :]
                    m = m_pool.tile([128, BL, 128], bf16)
                    nc.vector.tensor_mul(
                        m[:], nvrm[:], wo.to_broadcast([128, BL, GN])
                    )
                    nc.vector.tensor_reduce(
                        out=outacc[:, g, :], in_=m[:], axis=AX.X, op=ALU.add
                    )

            nc.sync.dma_start(out=outd[d], in_=outacc[:])

    nc.finalize()
    return nc


def _run_cores(nc, prep, embedding, n_cores):
    from concourse import bass_utils

    in_maps = []
    base = {
        "pidx": prep["pidx"],
        "newt": prep["newt"],
        "kvecc": prep["kvec"],
        "wcat": prep["wcat"],
        "b1c": prep["b1"],
        "b2c": prep["b2"],
    }
    for core in range(n_cores):
        eb = embedding[core * BL : (core + 1) * BL]  # [BL, H]
        flat = eb.reshape(-1)
        nvinit = np.zeros((2, ROW), np.float32)
        nvinit[1] = flat
        m = dict(base)
        m["nvinit"] = np.ascontiguousarray(nvinit.astype(BF16))
        m["rootnm"] = np.ascontiguousarray(
            np.broadcast_to(flat[None, :], (128, ROW)).astype(BF16)
        )
        in_maps.append(m)
    res = bass_utils.run_bass_kernel_spmd(
        nc, in_maps, core_ids=list(range(n_cores))
    )
    global LAST_RESULTS
    LAST_RESULTS = res
    return res


def _assemble(results, prep, inputs, n_cores):
    embedding = np.asarray(inputs["embedding"], np.float32)
    Wout = np.asarray(inputs["Wout"], np.float32)
    bout = np.asarray(inputs["bout"], np.float32)
    NTOT = 1 + D * P

    out = np.empty((embedding.shape[0], NTOT), np.float32)
    out[:, 0] = embedding @ Wout[0] + bout[0]
    for core in range(n_cores):
        v = results[core]["outd"]  # [D, 128, G, BL] (d, p, g, b)
        for d in range(D):
            flat = v[d].transpose(2, 1, 0).reshape(BL, PP)  # [b, g*128+p]
            cols = 1 + d * P + prep["perms"][d]
            out[core * BL : (core + 1) * BL, cols] = flat[:, :P]
    out[:, 1:] += bout[None, 1:NTOT]
    return out


def kernel(**inputs):
    prep = _prepare(inputs)
    nc = _build(prep)
    res = _run_cores(
        nc, prep, np.asarray(inputs["embedding"], np.float32), NCORES
    )
    return _assemble(res.results, prep, inputs, NCORES)
